# revision 26
# baseline (speedup 1.0000x reference)
"""Causal self-attention with RoPE on 8 Trainium2 NeuronCores.

Sharding: core c handles batch b = c//2 and head-group g = c%2 (8 of the 16
heads).  Wq/Wk/Wv are column-sharded (per head group), Wp is row-sharded;
each core computes a partial output projection for its batch and the host
sums the two partials per batch (the row-parallel unshard).

Device layouts (per core):
  xT    [C=1024, T=2048]  x transposed (contraction-friendly)
  wqT/wkT/wvT [1024, 512] W shard transposed ([c, d_local])
  wpT   [512, 1024]       Wp shard transposed ([c_local, e])
  cosT/sinT [128, 2048]   RoPE tables in [d, t] layout (2 head replicas,
                          sign folded into sinT for the rotate-half term)
  out   [2048, 1024] f32  partial projection output

Inside: q^T,k^T computed in [d, t] layout, v in [t, d]; scores computed
transposed (S^T = [k, t_q]) so softmax-normalizer and attention-output both
come from plain matmuls (V gets an appended ones-column to produce the
softmax denominator for free); causal mask applied post-exp via
affine_select (exact zeros).  All matmuls bf16 with fp32 PSUM accumulate.

Schedule: one global filler queue of projection/output-piece generators.
Window-0 attention interleaves into the projection stream from the first
matmul; every later pair pulls filler between its score blocks so the PE
never drains while ACT streams exps (keeps HAM at full clock).  The softmax
normalizer broadcast runs as a K=1 outer-product matmul on the PE instead
of gpsimd partition_broadcast, keeping the gpsimd FIFO free for the
causal-mask selects that sit on the exp->AV critical path.  The last four
output-projection tiles are split so only the pair-3 contraction step runs
after the final epilogue.
"""

import os
import sys

sys.path.insert(0, "/opt/trn_rl_repo")

from collections import deque

import ml_dtypes
import numpy as np

import concourse.bass as bass
import concourse.mybir as mybir
import concourse.tile as tile
from concourse import bacc
from concourse.bass_utils import run_bass_kernel_spmd

BF = mybir.dt.bfloat16
F32 = mybir.dt.float32
F32R = mybir.dt.float32r
NPBF = ml_dtypes.bfloat16

B, T, C = 4, 2048, 1024
H, D = 16, 64
HL, DL = 8, 512  # heads / channels per core
NCT = C // 128  # 8 contraction tiles
NTT = T // 512  # 4 big time windows
NT16 = T // 128  # 16 small time windows
ROPE_BASE = 10000.0

SWAP_MASK = [i ^ 1 for i in range(32)]


def _build_nc():
    nc = bacc.Bacc("TRN2", target_bir_lowering=False, debug=False)

    xT_d = nc.dram_tensor("xT", [C, T], BF, kind="ExternalInput")
    wq_d = nc.dram_tensor("wqT", [C, DL], BF, kind="ExternalInput")
    wk_d = nc.dram_tensor("wkT", [C, DL], BF, kind="ExternalInput")
    wv_d = nc.dram_tensor("wvT", [C, DL], BF, kind="ExternalInput")
    wp_d = nc.dram_tensor("wpT", [DL, C], BF, kind="ExternalInput")
    cos_d = nc.dram_tensor("cosT", [128, T], BF, kind="ExternalInput")
    sin_d = nc.dram_tensor("sinT", [128, T], BF, kind="ExternalInput")
    out_d = nc.dram_tensor("out", [T, C], BF, kind="ExternalOutput")

    with tile.TileContext(nc) as tc:
        _body(nc, tc, xT_d, wq_d, wk_d, wv_d, wp_d, cos_d, sin_d, out_d)
    nc.compile()
    return nc


def _body(nc, tc, xT_d, wq_d, wk_d, wv_d, wp_d, cos_d, sin_d, out_d):
    import contextlib

    ctx = contextlib.ExitStack()
    with ctx:
        const = ctx.enter_context(tc.tile_pool(name="const", bufs=1))
        work = ctx.enter_context(tc.tile_pool(name="work", bufs=2))
        psum = ctx.enter_context(tc.tile_pool(name="psum", bufs=1, space="PSUM"))

        # ---- resident SBUF tensors -------------------------------------
        x_sb = const.tile([128, NCT, T], BF)
        wq_sb = const.tile([128, NCT, DL], BF)
        wk_sb = const.tile([128, NCT, DL], BF)
        wv_sb = const.tile([128, NCT, DL], BF)
        x_dr = xT_d[:].rearrange("(a p) t -> p a t", p=128)
        wq_dr = wq_d[:].rearrange("(a p) d -> p a d", p=128)
        wk_dr = wk_d[:].rearrange("(a p) d -> p a d", p=128)
        wv_dr = wv_d[:].rearrange("(a p) d -> p a d", p=128)
        cos_sb = const.tile([128, T], BF)
        sin_sb = const.tile([128, T], BF)
        # loads in consumption order, chunky transfers (descriptor-count
        # matters: the sync queue serializes dma_starts).  q/k projections
        # for window 0 consume x[w0]+wq+wk first; v projections follow, so
        # wv streams after; the rest trails behind.
        for ct in range(NCT):
            nc.sync.dma_start(out=x_sb[:, ct, 0:512], in_=x_dr[:, ct, 0:512])
            nc.sync.dma_start(out=wq_sb[:, ct, :], in_=wq_dr[:, ct, :])
            nc.sync.dma_start(out=wk_sb[:, ct, :], in_=wk_dr[:, ct, :])
            if ct == 1:
                nc.sync.dma_start(out=cos_sb[:, 0:512], in_=cos_d[:, 0:512])
                nc.sync.dma_start(out=sin_sb[:, 0:512], in_=sin_d[:, 0:512])
        for ct in range(NCT):
            nc.sync.dma_start(out=wv_sb[:, ct, :], in_=wv_dr[:, ct, :])
        for ct in range(NCT):
            nc.sync.dma_start(out=x_sb[:, ct, 512:1024], in_=x_dr[:, ct, 512:1024])
        nc.sync.dma_start(out=cos_sb[:, 512:], in_=cos_d[:, 512:])
        nc.sync.dma_start(out=sin_sb[:, 512:], in_=sin_d[:, 512:])
        for ct in range(NCT):
            nc.sync.dma_start(out=x_sb[:, ct, 1024:2048], in_=x_dr[:, ct, 1024:2048])
        wp_sb = const.tile([128, 4, C], BF)
        nc.sync.dma_start(out=wp_sb, in_=wp_d[:].rearrange("(a p) e -> p a e", p=128))

        # v in [t, h, d(+ones)] layout; col 64 of each head group is 1.0
        v_sb = const.tile([128, NT16, HL, 65], BF)
        nc.vector.memset(v_sb[:, :, :, 64], 1.0)

        qr_sb = const.tile([128, 4, T], BF)  # q^T after rope, 4 head-pair tiles
        kr_sb = const.tile([128, 4, T], BF)
        yT_sb = const.tile([128, 4, T], BF)  # attention out, pre-projection

        # ---- piece generators ------------------------------------------
        def rope_evac(ps, tsl, nm, late=False):
            # ACT copy frees the pj PSUM slot without queuing behind the DVE
            # backlog; but window 3's pieces run while ACT is exp-saturated
            # (window 2), so those evacuate via DVE instead.
            ev = work.tile([128, 512], BF, tag="ev", bufs=2, name=f"ev{nm}")
            if late:
                nc.vector.tensor_copy(ev, ps)
            else:
                nc.scalar.copy(ev, ps)
            sh = work.tile([128, 512], BF, tag="sh", bufs=2, name=f"sh{nm}")
            nc.vector.stream_shuffle(sh, ev, SWAP_MASK)
            t1 = work.tile([128, 512], BF, tag="t1", bufs=3, name=f"t1{nm}")
            nc.vector.tensor_mul(t1, sh, sin_sb[:, tsl])
            t2 = work.tile([128, 512], BF, tag="t2", bufs=3, name=f"t2{nm}")
            nc.vector.tensor_mul(t2, ev, cos_sb[:, tsl])
            return t1, t2

        def qk_piece(tt, m):
            """Generator: q^T,k^T (+rope) for pair m, window tt."""
            tsl = slice(tt * 512, (tt + 1) * 512)
            dsl = slice(m * 128, (m + 1) * 128)
            for W, dst, nm in ((wq_sb, qr_sb, "q"), (wk_sb, kr_sb, "k")):
                ps = psum.tile([128, 512], F32, tag="pj", bufs=2, name=f"{nm}p{tt}_{m}")
                for ct in range(NCT):
                    nc.tensor.matmul(
                        ps,
                        lhsT=W[:, ct, dsl],
                        rhs=x_sb[:, ct, tsl],
                        start=(ct == 0),
                        stop=(ct == NCT - 1),
                    )
                    yield
                t1, t2 = rope_evac(ps, tsl, f"{nm}{tt}_{m}", late=(tt == 3))
                nc.vector.tensor_add(dst[:, m, tsl], t1, t2)
                yield

        def v_piece(t16):
            """Generator: v for small time window t16."""
            vp = psum.tile([128, 512], F32, tag="pj", bufs=2, name=f"vp{t16}")
            for ct in range(NCT):
                nc.tensor.matmul(
                    vp,
                    lhsT=x_sb[:, ct, t16 * 128 : (t16 + 1) * 128],
                    rhs=wv_sb[:, ct, :],
                    start=(ct == 0),
                    stop=(ct == NCT - 1),
                )
                yield
            vpr = vp.rearrange("p (h d) -> p h d", h=HL)
            if t16 >= 12:
                nc.vector.tensor_copy(v_sb[:, t16, :, 0:64], vpr)
            else:
                nc.scalar.copy(v_sb[:, t16, :, 0:64], vpr)
            yield

        def out_piece(t16):
            """Generator: output-projection partial for time window t16."""
            t16sl = slice(t16 * 128, (t16 + 1) * 128)
            osb = work.tile([128, C], BF, tag="osb", bufs=3, name=f"osb{t16}")
            for e2 in (0, 1):
                op = psum.tile([128, 512], F32, tag="pj", bufs=2, name=f"op{t16}_{e2}")
                for hdt in range(4):
                    nc.tensor.matmul(
                        op,
                        lhsT=yT_sb[:, hdt, t16sl],
                        rhs=wp_sb[:, hdt, e2 * 512 : (e2 + 1) * 512],
                        start=(hdt == 0),
                        stop=(hdt == 3),
                    )
                    yield
                nc.vector.tensor_copy(osb[:, e2 * 512 : (e2 + 1) * 512], op)
                yield
            nc.sync.dma_start(out=out_d[t16sl, :], in_=osb)
            yield

        # final four output tiles: pairs 0-1 accumulate early (A); the
        # pair-2/3 contraction steps + combine + DMA (B) are all that runs
        # after the last two epilogues.
        osbA = {}

        def outA_piece(t16):
            t16sl = slice(t16 * 128, (t16 + 1) * 128)
            for e2 in (0, 1):
                op = psum.tile([128, 512], F32, tag="pj", bufs=2, name=f"oa{t16}_{e2}")
                for hdt in range(2):
                    nc.tensor.matmul(
                        op,
                        lhsT=yT_sb[:, hdt, t16sl],
                        rhs=wp_sb[:, hdt, e2 * 512 : (e2 + 1) * 512],
                        start=(hdt == 0),
                        stop=(hdt == 1),
                    )
                    yield
                pa = work.tile([128, 512], BF, tag="osbA", bufs=8, name=f"pa{t16}_{e2}")
                nc.vector.tensor_copy(pa, op)
                osbA[(t16, e2)] = pa
                yield

        def tailB_piece(t16):
            t16sl = slice(t16 * 128, (t16 + 1) * 128)
            osb = work.tile([128, C], BF, tag="osb", bufs=3, name=f"osbB{t16}")
            for e2 in (0, 1):
                op = psum.tile([128, 512], F32, tag="pj", bufs=2, name=f"ob{t16}_{e2}")
                for hdt in (2, 3):
                    nc.tensor.matmul(
                        op,
                        lhsT=yT_sb[:, hdt, t16sl],
                        rhs=wp_sb[:, hdt, e2 * 512 : (e2 + 1) * 512],
                        start=(hdt == 2),
                        stop=(hdt == 3),
                    )
                    yield
                nc.vector.tensor_add(
                    osb[:, e2 * 512 : (e2 + 1) * 512], osbA[(t16, e2)], op
                )
                yield
            nc.sync.dma_start(out=out_d[t16sl, :], in_=osb)
            yield

        # ---- global filler queue ---------------------------------------
        class FQ:
            def __init__(self):
                self.q = deque()

            def add(self, g):
                self.q.append(g)

            def pull(self, n):
                k = 0
                while self.q and k < n:
                    try:
                        next(self.q[0])
                        k += 1
                    except StopIteration:
                        self.q.popleft()

            def force(self, g):
                try:
                    self.q.remove(g)
                except ValueError:
                    pass
                for _ in g:
                    pass

            def drain_all(self):
                while self.q:
                    g = self.q.popleft()
                    for _ in g:
                        pass

        def attention(m, qt, fq, boosted=False, inject=None):
            """Both heads of pair m, query window qt (row-packed on PE).

            All S^T matmuls are issued first (exps stream behind on ACT),
            with filler pulled from fq between score tiles; the AV matmuls
            run one block behind, by which time every exp has finished —
            the PE never waits on the scalar engine."""
            qsl = slice(qt * 512, (qt + 1) * 512)
            nk = 4 * qt + 4
            yxs = [
                psum.tile([65, 512], F32, tag="yx", bufs=2, name=f"yx{m}_{qt}_{h2}")
                for h2 in (0, 1)
            ]

            def emit_st(ki):
                # one [128,1024] tile: head A scores in cols 0-511 (bank 1),
                # head B in cols 512-1023 (bank 2); the two matmuls run
                # concurrently in disjoint PE row groups (K=64 each).
                # Diagonal k-tiles only compute the live (unmasked) q-range
                # [q0, 512) — q columns below 128*(ki-4qt) are fully masked.
                q0 = max(0, 128 * ki - 512 * qt)
                w = 512 - q0
                st = psum.tile([128, 1024], F32, tag="st", bufs=2, name=f"st{m}_{qt}_{ki}")
                for h2 in (0, 1):
                    rsl = slice(64 * h2, 64 * h2 + 64)
                    nc.tensor.matmul(
                        st[:, h2 * 512 + q0 : (h2 + 1) * 512],
                        lhsT=kr_sb[rsl, m, ki * 128 : (ki + 1) * 128],
                        rhs=qr_sb[rsl, m, qt * 512 + q0 : (qt + 1) * 512],
                        start=True,
                        stop=True,
                    )
                pt = work.tile([128, 1024], BF, tag="pt", bufs=12, name=f"pt{m}_{qt}_{ki}")
                stv = st.rearrange("p (g c) -> p g c", g=2)[:, :, q0:512]
                ptv = pt.rearrange("p (g c) -> p g c", g=2)[:, :, q0:512]
                nc.scalar.activation(
                    ptv, stv, mybir.ActivationFunctionType.Exp, scale=0.125
                )
                if ki >= 4 * qt:  # diagonal block: causal mask (both halves)
                    nc.gpsimd.affine_select(
                        ptv,
                        ptv,
                        pattern=[[0, 2], [1, w]],
                        compare_op=mybir.AluOpType.is_ge,
                        fill=0.0,
                        base=0,
                        channel_multiplier=-1,
                    )
                return pt

            def emit_av(ki, pt):
                q0 = max(0, 128 * ki - 512 * qt)
                for h2 in (0, 1):
                    nc.tensor.matmul(
                        yxs[h2][:, q0:512],
                        lhsT=v_sb[:, ki, 2 * m + h2, :],
                        rhs=pt[:, h2 * 512 + q0 : (h2 + 1) * 512],
                        start=(ki == 0),
                        stop=(ki == nk - 1),
                    )

            # software-pipelined blocks of k-tiles: block j's AVs issue
            # only after block j+1's scores, so every exp has a full block
            # of PE work (scores + filler) between its score matmul and the
            # AV that consumes it.  Window 0 (nk=4) halves its block size
            # so it still gets a score->AV pipeline lag.
            blk = 2 if nk <= 4 else 4
            pending = None
            nblk = 0
            for k0 in range(0, nk, blk):
                kis = list(range(k0, min(k0 + blk, nk)))
                pts = []
                if boosted and nblk == 0:
                    fq.pull(4)  # pre-cover: first score may wait on rope
                for ki in kis:
                    pts.append(emit_st(ki))
                    fq.pull(6 if (boosted and nblk < 2) else 3)
                if nblk == 0 and inject:
                    for g in inject:
                        fq.force(g)
                if pending is not None:
                    for ki, pt in zip(*pending):
                        emit_av(ki, pt)
                pending = (kis, pts)
                nblk += 1
            for ki, pt in zip(*pending):
                emit_av(ki, pt)

            # evacuate PSUM immediately for BOTH heads (frees the yx banks
            # fast); the normalizer rows leave as f32 at partition 0 (the
            # custom-DVE recip only behaves at base 0) so the reciprocal
            # can run on them directly.  The slow division chain is
            # deferred.
            ysrs, dens = [], []
            for h2 in (0, 1):
                ysr = work.tile(
                    [64, 512], BF, tag="ysr", bufs=6, name=f"ysr{m}_{qt}_{h2}"
                )
                nc.vector.tensor_copy(ysr, yxs[h2][0:64, :])
                den = work.tile(
                    [1, 512], F32, tag=f"den{h2}", bufs=2, name=f"den{m}_{qt}_{h2}"
                )
                nc.vector.tensor_copy(den, yxs[h2][64:65, :])
                ysrs.append(ysr)
                dens.append(den)

            def epilogue():
                # per-head chain: recip (DVE, base 0, straight off the f32
                # normalizer row) -> gpsimd partition_broadcast -> mul.
                # The cross-window stagger pops this a full pair later, so
                # the broadcasts land mid-window where the gpsimd FIFO is
                # quiet instead of ahead of the next window's mask selects.
                for h2 in (0, 1):
                    rl = work.tile(
                        [1, 512], F32, tag=f"rl{h2}", bufs=1, name=f"rl{m}_{qt}_{h2}"
                    )
                    nc.vector.reciprocal_approx_fast(rl, dens[h2])
                    rlb = work.tile(
                        [64, 512], F32, tag="rlb", bufs=2, name=f"rlb{m}_{qt}_{h2}"
                    )
                    nc.gpsimd.partition_broadcast(rlb, rl)
                    rsl = slice(64 * h2, 64 * h2 + 64)
                    nc.vector.tensor_mul(yT_sb[rsl, m, qsl], ysrs[h2], rlb)

            return epilogue

        # ---- merged schedule ------------------------------------------
        fq = FQ()
        qk = {(tt, m): qk_piece(tt, m) for tt in range(NTT) for m in range(4)}
        vs = {t16: v_piece(t16) for t16 in range(NT16)}
        outs = {t16: out_piece(t16) for t16 in range(12)}
        outsA = {t16: outA_piece(t16) for t16 in range(12, 16)}
        tailsB = {t16: tailB_piece(t16) for t16 in range(12, 16)}

        fq.force(qk[(0, 0)])
        for m in (1, 2, 3):
            fq.add(qk[(0, m)])
        for i in range(4):
            fq.add(vs[i])
        for m in range(4):
            fq.add(qk[(1, m)])
        for i in range(4, 8):
            fq.add(vs[4 + (i - 4)])

        pend = deque()

        def pop_ep():
            ep, ptt, pm = pend.popleft()
            ep()
            if pm == 3 and ptt <= 2:
                # window ptt's last epilogue done -> its output-projection
                # pieces are now legal filler
                for i in range(4):
                    fq.add(outs[4 * ptt + i])

        for tt in range(NTT):
            if 1 <= tt <= 2:
                for m2 in range(4):
                    fq.add(qk[(tt + 1, m2)])
                for i in range(4):
                    fq.add(vs[4 * (tt + 1) + i])
            for m in range(4):
                if (tt, m) != (0, 0):
                    fq.force(qk[(tt, m)])
                inject = [vs[4 * tt + i] for i in range(4)] if m == 0 else None
                ep = attention(m, tt, fq, boosted=(m == 0), inject=inject)
                pend.append((ep, tt, m))
                if len(pend) > 1:
                    pop_ep()
                if tt == NTT - 1 and m == 2:
                    # eps (qt=3, pairs 0-1) have popped -> A-phases legal
                    for t16 in range(12, 16):
                        fq.add(outsA[t16])
            # boundary cover: emit dep-free filler right where the next
            # window's first scores will wait out the ACT exp backlog
            fq.pull(24)

        # tail: drain leftovers while the pair-2/3 epilogue chains run,
        # then the last two output-projection contraction steps
        fq.drain_all()
        while pend:
            pop_ep()
        for t16 in range(12, 16):
            for _ in tailsB[t16]:
                pass


_NC_CACHE = None
LAST_RESULT = None


def _get_nc():
    global _NC_CACHE
    if _NC_CACHE is None:
        _NC_CACHE = _build_nc()
    return _NC_CACHE


def _rope_tables(start_pos):
    inv = 1.0 / (ROPE_BASE ** (np.arange(0, D, 2, dtype=np.float32) / D))
    t = np.arange(T, dtype=np.float32) + np.float32(start_pos)
    freqs = t[:, None] * inv[None, :]  # [T, 32]
    emb = np.concatenate([freqs, freqs], axis=-1)  # [T, 64]
    cos = np.cos(emb).T  # [64, T]
    sin = np.sin(emb).T
    sgn = np.where(np.arange(D) % 2 == 0, -1.0, 1.0).astype(np.float32)
    cosT = np.tile(cos, (2, 1))
    sinT = np.tile(sin * sgn[:, None], (2, 1))
    return cosT.astype(NPBF), sinT.astype(NPBF)


def kernel(x, Wq, Wk, Wv, Wp, start_pos):
    x = np.asarray(x, dtype=np.float32)
    Wq = np.asarray(Wq, dtype=np.float32)
    Wk = np.asarray(Wk, dtype=np.float32)
    Wv = np.asarray(Wv, dtype=np.float32)
    Wp = np.asarray(Wp, dtype=np.float32)
    cosT, sinT = _rope_tables(int(start_pos))

    nc = _get_nc()
    in_maps = []
    for c in range(8):
        b, g = divmod(c, 2)
        hs = slice(g * DL, (g + 1) * DL)
        in_maps.append(
            {
                "xT": np.ascontiguousarray(x[b].T).astype(NPBF),
                "wqT": np.ascontiguousarray(Wq[hs, :].T).astype(NPBF),
                "wkT": np.ascontiguousarray(Wk[hs, :].T).astype(NPBF),
                "wvT": np.ascontiguousarray(Wv[hs, :].T).astype(NPBF),
                "wpT": np.ascontiguousarray(Wp[:, hs].T).astype(NPBF),
                "cosT": cosT,
                "sinT": sinT,
            }
        )
    try:
        res = run_bass_kernel_spmd(nc, in_maps, core_ids=list(range(8)))
    except ModuleNotFoundError:
        # BASS_TRACE set but the axon NTFF hook module is unavailable in
        # this environment — rerun with tracing disabled.
        os.environ["BASS_NEVER_TRACE"] = "1"
        res = run_bass_kernel_spmd(nc, in_maps, core_ids=list(range(8)))
    global LAST_RESULT
    LAST_RESULT = res
    outs = [np.asarray(r["out"], dtype=np.float32) for r in res.results]
    full = np.stack(
        [outs[2 * b] + outs[2 * b + 1] for b in range(B)], axis=0
    )
    return full.astype(np.float32)


if __name__ == "__main__":
    nc = _get_nc()
    print("built ok")
